# revision 1
# baseline (speedup 1.0000x reference)
"""3D Haar DWT low-pass (DWT3DTiny) Trainium2 kernel.

The reference applies the Haar rec_lo filter [s, s] (s = sqrt(2)/2) with
stride-2 downsampling along t, h, w for every channel.  That is exactly a
2x2x2 box sum scaled by s^3 = 2**-1.5:

    out[ts, hs, ws, c] = 2**-1.5 * sum_{dt,dh,dw in {0,1}} x[2ts+dt, 2hs+dh, 2ws+dw, c]

Sharding: along t (pure data-parallel, t-pairs never cross a core
boundary since 32 / 8 = 4 rows per core), contiguous host-side slices.

Per-core kernel, measured ~97.5 us on HW per core (single-core and
8-core concurrent runs match).  The ~88 us steady state is HBM-limited:
36 MB of traffic at ~380 GB/s effective.  Design notes:
  * partition dim = 128 output g rows; each partition holds the h-row
    pair (2g, 2g+1) contiguously in its free dim;
  * per chunk the two t rows are loaded into separate tiles and
    h-reduced independently, so no compute instruction waits on more
    than one DMA semaphore (walrus allows 1 sync-wait per instruction);
  * all loads are issued on the SP HWDGE ring, all stores on the ACT
    ring - sharing one ring head-of-line blocks loads behind stores
    (~12 us slower);
  * the final chunk is split into 4 sub-chunks with their own small
    loads so the post-last-load pipeline drain is short (~4 us saved);
  * the dead const-tile memsets are stripped from the init preamble
    (~9 us of GpSimd startup the all-engine barrier otherwise waits on);
  * reduction chain per chunk: DVE h-add per t row, DVE t-add, DVE
    strided w-add, ACT scale by 2**-1.5.
Rejected experimentally: SWDGE accumulate loads (CCE add ~271 GB/s),
SWDGE plain-copy loads (Q7 descriptor-gen bound), 4 MB loads with
bufs=2 (pipeline starves), loads split across both HWDGE rings.
"""

import numpy as np

import concourse.bacc as bacc
import concourse.mybir as mybir
from concourse.bass_utils import run_bass_kernel_spmd
from concourse.tile import TileContext

N_CORES = 8
T, H, W, C = 32, 512, 512, 8
TS = T // N_CORES  # t rows per core
WI = 256  # input w per chunk
TAIL_SUB = 4  # sub-chunks for the final chunk
SCALE = float(2.0 ** -1.5)

_CACHE: dict = {}


def _build_nc() -> bacc.Bacc:
    nc = bacc.Bacc("TRN2", target_bir_lowering=False)
    x = nc.dram_tensor("x", [TS, H, W, C], mybir.dt.float32, kind="ExternalInput")
    y = nc.dram_tensor(
        "y", [TS // 2, H // 2, W // 2, C], mybir.dt.float32, kind="ExternalOutput"
    )

    # t = 2*tp + dt, h = gb*256 + p*2 + two  (g = gb*128 + p), w = u*WI + wi
    xq = x.rearrange(
        "t (gb p two) (u wi) c -> t gb u p two (wi c)", p=128, two=2, wi=WI
    )
    yq = y.rearrange("s (gb p) (u vi) c -> s gb u p (vi c)", p=128, vi=WI // 2)
    swi = WI // TAIL_SUB
    xs = x.rearrange(
        "t (gb p two) (u us swi) c -> t gb u us p two (swi c)",
        p=128, two=2, us=TAIL_SUB, swi=swi,
    )
    ys = y.rearrange(
        "s (gb p) (u us vi) c -> s gb u us p (vi c)", p=128, us=TAIL_SUB, vi=swi // 2
    )

    n_u = W // WI
    chunks = [
        (tp, gb, u)
        for tp in range(TS // 2)
        for gb in range(H // 256)
        for u in range(n_u)
    ]

    with TileContext(nc) as tc:
        with (
            tc.tile_pool(name="pin", bufs=3) as pin,
            tc.tile_pool(name="ph", bufs=3) as ph,
            tc.tile_pool(name="pw", bufs=3) as pw,
            tc.tile_pool(name="ptail", bufs=3) as pt,
        ):

            def chain(a, b, hw, hp, wp, tg, ydst):
                # h-pair within each t row (each waits on exactly one DMA)
                ha = hp.tile([128, hw], mybir.dt.float32, tag=tg + "ha")
                hb = hp.tile([128, hw], mybir.dt.float32, tag=tg + "hb")
                nc.vector.tensor_add(out=ha[:], in0=a[:, 0], in1=a[:, 1])
                nc.vector.tensor_add(out=hb[:], in0=b[:, 0], in1=b[:, 1])
                # t-pair (DVE-internal dependency only)
                nc.vector.tensor_add(out=ha[:], in0=ha[:], in1=hb[:])
                # w-pair (strided: wi = v*2 + dw)
                hv = ha.rearrange("p (v two c) -> p v two c", two=2, c=C)
                ws = wp.tile([128, hw // 2], mybir.dt.float32, tag=tg + "w")
                wv = ws.rearrange("p (v c) -> p v c", c=C)
                nc.vector.tensor_add(out=wv[:], in0=hv[:, :, 0], in1=hv[:, :, 1])
                nc.scalar.mul(ws[:], ws[:], SCALE)
                nc.scalar.dma_start(out=ydst, in_=ws[:])

            for ci, (tp, gb, u) in enumerate(chunks):
                if ci < len(chunks) - 1:
                    a = pin.tile([128, 2, WI * C], mybir.dt.float32, tag="a")
                    b = pin.tile([128, 2, WI * C], mybir.dt.float32, tag="b")
                    nc.sync.dma_start(out=a[:], in_=xq[2 * tp, gb, u])
                    nc.sync.dma_start(out=b[:], in_=xq[2 * tp + 1, gb, u])
                    chain(a, b, WI * C, ph, pw, "", yq[tp, gb, u])
                else:
                    for us in range(TAIL_SUB):
                        a = pt.tile([128, 2, swi * C], mybir.dt.float32, tag="ta")
                        b = pt.tile([128, 2, swi * C], mybir.dt.float32, tag="tb")
                        nc.sync.dma_start(out=a[:], in_=xs[2 * tp, gb, u, us])
                        nc.sync.dma_start(out=b[:], in_=xs[2 * tp + 1, gb, u, us])
                        chain(a, b, swi * C, pt, pt, "t", ys[tp, gb, u, us])

    _strip_init_preamble(nc)
    if not nc.is_finalized():
        nc.finalize()  # Bacc.compile: event-sem split (1 wait/inst), reg alloc
    return nc


def _strip_init_preamble(nc) -> None:
    """Drop the four Bass.__init__ const-tile memsets from block 0.  Nothing
    in this kernel reads the const tiles (scalar.mul uses an immediate), yet
    the initial all-engine barrier waits on the GpSimd engine executing them,
    which costs ~9 us of Q7 startup on HW.  The drains and the all-engine
    barrier themselves are kept intact."""
    b0 = nc.main_func.blocks[0]
    b0.instructions[:] = [
        ins for ins in b0.instructions if type(ins).__name__ != "InstMemset"
    ]


def kernel(x) -> np.ndarray:
    x = np.asarray(x, dtype=np.float32)
    assert x.shape == (T, H, W, C), x.shape

    if "nc" not in _CACHE:
        _CACHE["nc"] = _build_nc()
    nc = _CACHE["nc"]

    in_maps = [
        {"x": np.ascontiguousarray(x[i * TS : (i + 1) * TS])} for i in range(N_CORES)
    ]
    res = run_bass_kernel_spmd(nc, in_maps, core_ids=list(range(N_CORES)))
    return np.concatenate([r["y"] for r in res.results], axis=0)



# revision 2
# speedup vs baseline: 1.6972x; 1.6972x over previous
"""3D Haar DWT low-pass (DWT3DTiny) Trainium2 kernel.

The reference applies the Haar rec_lo filter [s, s] (s = sqrt(2)/2) with
stride-2 downsampling along t, h, w for every channel.  That is exactly a
2x2x2 box sum scaled by s^3 = 2**-1.5:

    out[ts, hs, ws, c] = 2**-1.5 * sum_{dt,dh,dw in {0,1}} x[2ts+dt, 2hs+dh, 2ws+dw, c]

Sharding: along t (pure data-parallel, t-pairs never cross a core
boundary since 32 / 8 = 4 rows per core), contiguous host-side slices.

The kernel is HBM/DMA-bound (~390 GB/s effective per core), so the input
is quantized host-side to fp16 (with the 2**-1.5 scale folded into the
cast) and the output is stored fp16 and upcast host-side.  This halves
the DMA traffic vs fp32: 16.8 MB load + 2.1 MB store per core.  The
rel-err budget (2e-2) dwarfs the fp16 quantization noise (~3e-4 rms).

Per-core design notes (inherited from the fp32 tuning):
  * partition dim = 128 output g rows; each partition holds the h-row
    pair (2g, 2g+1) contiguously in its free dim;
  * per chunk the two t rows are loaded into separate tiles and
    h-reduced independently, so no compute instruction waits on more
    than one DMA semaphore (walrus allows 1 sync-wait per instruction);
  * all loads are issued on the SP HWDGE ring, all stores on the ACT
    ring - sharing one ring head-of-line blocks loads behind stores;
  * the final chunk is split into 4 sub-chunks with their own small
    loads so the post-last-load pipeline drain is short;
  * the dead const-tile memsets are stripped from the init preamble
    (~9 us of GpSimd startup the all-engine barrier otherwise waits on);
  * reduction chain per chunk: DVE h-add per t row, DVE t-add, DVE
    strided w-add (scale already folded into the host-side quant).
Rejected experimentally (fp32 era): SWDGE accumulate loads, SWDGE
plain-copy loads, 4 MB loads with bufs=2, loads split across both
HWDGE rings.
"""

import numpy as np

import concourse.bacc as bacc
import concourse.mybir as mybir
from concourse.bass_utils import run_bass_kernel_spmd
from concourse.tile import TileContext

N_CORES = 8
T, H, W, C = 32, 512, 512, 8
TS = T // N_CORES  # t rows per core
WI = 256  # input w per chunk
TAIL_SUB = 4  # sub-chunks for the final chunk
SCALE = float(2.0 ** -1.5)
DT = mybir.dt.float16

_CACHE: dict = {}


def _build_nc() -> bacc.Bacc:
    nc = bacc.Bacc("TRN2", target_bir_lowering=False)
    x = nc.dram_tensor("x", [TS, H, W, C], DT, kind="ExternalInput")
    y = nc.dram_tensor("y", [TS // 2, H // 2, W // 2, C], DT, kind="ExternalOutput")

    # t = 2*tp + dt, h = gb*256 + p*2 + two  (g = gb*128 + p), w = u*WI + wi
    xq = x.rearrange(
        "t (gb p two) (u wi) c -> t gb u p two (wi c)", p=128, two=2, wi=WI
    )
    yq = y.rearrange("s (gb p) (u vi) c -> s gb u p (vi c)", p=128, vi=WI // 2)
    swi = WI // TAIL_SUB
    xs = x.rearrange(
        "t (gb p two) (u us swi) c -> t gb u us p two (swi c)",
        p=128, two=2, us=TAIL_SUB, swi=swi,
    )
    ys = y.rearrange(
        "s (gb p) (u us vi) c -> s gb u us p (vi c)", p=128, us=TAIL_SUB, vi=swi // 2
    )

    n_u = W // WI
    chunks = [
        (tp, gb, u)
        for tp in range(TS // 2)
        for gb in range(H // 256)
        for u in range(n_u)
    ]

    with TileContext(nc) as tc:
        with (
            tc.tile_pool(name="pin", bufs=3) as pin,
            tc.tile_pool(name="ph", bufs=3) as ph,
            tc.tile_pool(name="pw", bufs=3) as pw,
            tc.tile_pool(name="ptail", bufs=3) as pt,
        ):

            def chain(a, b, hw, hp, wp, tg, ydst):
                # h-pair within each t row (each waits on exactly one DMA)
                ha = hp.tile([128, hw], DT, tag=tg + "ha")
                hb = hp.tile([128, hw], DT, tag=tg + "hb")
                nc.vector.tensor_add(out=ha[:], in0=a[:, 0], in1=a[:, 1])
                nc.vector.tensor_add(out=hb[:], in0=b[:, 0], in1=b[:, 1])
                # t-pair (DVE-internal dependency only)
                nc.vector.tensor_add(out=ha[:], in0=ha[:], in1=hb[:])
                # w-pair (strided: wi = v*2 + dw)
                hv = ha.rearrange("p (v two c) -> p v two c", two=2, c=C)
                ws = wp.tile([128, hw // 2], DT, tag=tg + "w")
                wv = ws.rearrange("p (v c) -> p v c", c=C)
                nc.vector.tensor_add(out=wv[:], in0=hv[:, :, 0], in1=hv[:, :, 1])
                nc.scalar.dma_start(out=ydst, in_=ws[:])

            for ci, (tp, gb, u) in enumerate(chunks):
                if ci < len(chunks) - 1:
                    a = pin.tile([128, 2, WI * C], DT, tag="a")
                    b = pin.tile([128, 2, WI * C], DT, tag="b")
                    nc.sync.dma_start(out=a[:], in_=xq[2 * tp, gb, u])
                    nc.sync.dma_start(out=b[:], in_=xq[2 * tp + 1, gb, u])
                    chain(a, b, WI * C, ph, pw, "", yq[tp, gb, u])
                else:
                    for us in range(TAIL_SUB):
                        a = pt.tile([128, 2, swi * C], DT, tag="ta")
                        b = pt.tile([128, 2, swi * C], DT, tag="tb")
                        nc.sync.dma_start(out=a[:], in_=xs[2 * tp, gb, u, us])
                        nc.sync.dma_start(out=b[:], in_=xs[2 * tp + 1, gb, u, us])
                        chain(a, b, swi * C, pt, pt, "t", ys[tp, gb, u, us])

    _strip_init_preamble(nc)
    if not nc.is_finalized():
        nc.finalize()  # Bacc.compile: event-sem split (1 wait/inst), reg alloc
    return nc


def _strip_init_preamble(nc) -> None:
    """Drop the four Bass.__init__ const-tile memsets from block 0.  Nothing
    in this kernel reads the const tiles, yet the initial all-engine barrier
    waits on the GpSimd engine executing them, which costs ~9 us of Q7
    startup on HW.  The drains and the all-engine barrier are kept intact."""
    b0 = nc.main_func.blocks[0]
    b0.instructions[:] = [
        ins for ins in b0.instructions if type(ins).__name__ != "InstMemset"
    ]


def kernel(x) -> np.ndarray:
    x = np.asarray(x, dtype=np.float32)
    assert x.shape == (T, H, W, C), x.shape

    if "nc" not in _CACHE:
        _CACHE["nc"] = _build_nc()
    nc = _CACHE["nc"]

    # Quantize to fp16 with the 2**-1.5 wavelet scale folded into the cast
    # (the device kernel is then a pure 2x2x2 box sum).
    in_maps = [
        {"x": (x[i * TS : (i + 1) * TS] * SCALE).astype(np.float16)}
        for i in range(N_CORES)
    ]
    res = run_bass_kernel_spmd(nc, in_maps, core_ids=list(range(N_CORES)))
    return np.concatenate([r["y"] for r in res.results], axis=0).astype(np.float32)
